# revision 16
# baseline (speedup 1.0000x reference)
"""Trainium2 Bass kernel: EdgeFeatureEncoding scatter-add (raw bass).

Computes bias[i, j, :] += edge_attr[e] @ W + b over E edges (i, j),
bias shape (N, N, 8) with N = 4096, E = 131072 -> 512 MiB f32 output.

Strategy (8 NeuronCores, SPMD, hand-rolled semaphores):
- Output rows i are sharded across the 8 cores (512 rows -> 64 MiB each).
- Each shard splits into 16 ZONES with per-zone chunk counts (max over
  cores, so one compiled program fits all cores).
- One HWDGE ring (sync engine) carries, in FIFO order, the constants then
  interleaved (x_z, zero_z) transfers: edge features stream in just ahead
  of each zone's zero-fill, so compute leads the scatter gate.
- Edge features ship pre-transposed ([feat, edge]), so the projection is
  ONE PE matmul per 128-edge chunk straight into [128 edges, 8 heads]
  PSUM (lhsT = xT chunk, rhs = W); DVE adds the bias into the scatter
  source buffer.  No transposes, no PSUM->SBUF relays.
- GpSimd scatters each chunk with one indirect DMA (one dest row per
  partition - HW semantics).  A zone's scatters wait ONLY on that zone's
  own zero-fill semaphore: zero-fill, compute and scatter all pipeline.
- Each zone's chunk 0 carries every duplicate-destination group (plus
  singleton filler); the device group-sums it with the
  is_equal/selection-matrix matmul, so colliding DMA writes all carry the
  identical group sum (singletons pass through the selection matmul
  unchanged).
- Table rows [0, 128) are a trash target for padding edges (sliced off on
  the host); real row d lives at table row 128 + d.
"""

import os
from dataclasses import dataclass

import numpy as np

H = 8  # n_heads
F = 128  # edge feature dim
CH = 128  # edges per chunk (one partition tile / one indirect DMA)
TRASH = 128  # trash rows at the START of the table
N_CORES = 8
ZONES = 16  # zero-fill zones per core
ZSPLIT = 2  # zero-fill DMAs per zone


@dataclass(frozen=True)
class _Cfg:
    n_nodes: int
    n_shards: int
    quotas: tuple  # chunks per zone (chunk 0 of each zone = selection chunk)

    @property
    def rows(self):
        return self.n_nodes // self.n_shards

    @property
    def table_real(self):
        return self.rows * self.n_nodes

    @property
    def zone_rows(self):
        return self.table_real // ZONES

    @property
    def table_rows(self):
        return TRASH + self.table_real


_cache: dict = {}


def _build(cfg: _Cfg):
    import concourse.bacc as bacc
    import concourse.bass as bass
    import concourse.mybir as mybir

    f32 = mybir.dt.float32
    i32 = mybir.dt.int32
    quotas = cfg.quotas
    NCH = sum(quotas)  # total chunks
    ofs = [0]
    for q in quotas:
        ofs.append(ofs[-1] + q)

    nc = bacc.Bacc(
        "TRN2", target_bir_lowering=False, debug=False, num_devices=cfg.n_shards
    )
    # xt[f, (ofs[z] + c)*CH + p] = feature f of edge (zone z, chunk c, row p)
    xt = nc.dram_tensor("xt", [F, NCH * CH], f32, kind="ExternalInput")
    # idxb[p, ofs[z] + c] = dest table row of edge (zone z, chunk c, row p)
    idxb = nc.dram_tensor("idxb", [CH, NCH], i32, kind="ExternalInput")
    w = nc.dram_tensor("w", [F, H], f32, kind="ExternalInput")
    brep = nc.dram_tensor("brep", [CH, H], f32, kind="ExternalInput")
    table = nc.dram_tensor("table", [cfg.table_rows, H], f32, kind="ExternalOutput")

    zcols = cfg.zone_rows * H // (128 * ZSPLIT)  # f32/partition per zero DMA
    zview = table.ap()[TRASH:].rearrange(
        "(zz p x) h -> zz p (x h)", zz=ZONES * ZSPLIT, p=128
    )

    # ---- SBUF / PSUM ----
    ztile = nc.alloc_sbuf_tensor("ztile", [128, zcols], f32)
    wt = nc.alloc_sbuf_tensor("wt", [F, H], f32)
    bt = nc.alloc_sbuf_tensor("bt", [CH, H], f32)
    ixt = nc.alloc_sbuf_tensor("ixt", [CH, NCH], i32)
    ident = nc.alloc_sbuf_tensor("ident", [CH, CH], f32)
    xz = [
        nc.alloc_sbuf_tensor(f"xz{z}", [F, quotas[z] * CH], f32) for z in range(ZONES)
    ]
    srcb = nc.alloc_sbuf_tensor("srcb", [CH, NCH * H], f32)
    idxf = [nc.alloc_sbuf_tensor(f"idxf{i}", [CH, 1], f32) for i in range(2)]
    idt_sb = [nc.alloc_sbuf_tensor(f"idt{i}", [CH, CH], f32) for i in range(2)]
    selm = [nc.alloc_sbuf_tensor(f"selm{i}", [CH, CH], f32) for i in range(2)]
    pj_sb = [nc.alloc_sbuf_tensor(f"pjsb{i}", [CH, H], f32) for i in range(2)]

    pj_ps = [nc.alloc_psum_tensor(f"pj{i}", [CH, H], f32) for i in range(4)]
    idt_ps = nc.alloc_psum_tensor("idtp", [CH, CH], f32)
    acc_ps = nc.alloc_psum_tensor("accp", [CH, H], f32)

    # ---- semaphores ----
    s_zt = nc.alloc_semaphore("s_zt")
    s_w = nc.alloc_semaphore("s_w")
    s_b = nc.alloc_semaphore("s_b")
    s_ix = nc.alloc_semaphore("s_ix")
    s_x = [nc.alloc_semaphore(f"s_x{z}") for z in range(ZONES)]
    s_z = [nc.alloc_semaphore(f"s_z{z}") for z in range(ZONES)]
    s_id = nc.alloc_semaphore("s_id")
    s_mm = nc.alloc_semaphore("s_mm")
    s_src = nc.alloc_semaphore("s_src")
    s_idxf = nc.alloc_semaphore("s_idxf")
    s_idt = nc.alloc_semaphore("s_idt")
    s_idtcp = nc.alloc_semaphore("s_idtcp")
    s_selv = nc.alloc_semaphore("s_selv")
    s_selmm = nc.alloc_semaphore("s_selmm")
    s_sc = nc.alloc_semaphore("s_sc")

    # ---- SYNC: constants, then (x_z, zero_z) interleaved on one ring ----
    sy = nc.sync
    sy.dma_start(out=wt.ap(), in_=w.ap()).then_inc(s_w, 16)
    sy.dma_start(out=bt.ap(), in_=brep.ap()).then_inc(s_b, 16)
    sy.dma_start(out=ixt.ap(), in_=idxb.ap()).then_inc(s_ix, 16)
    sy.wait_ge(s_zt, 1)
    for z in range(ZONES):
        sy.dma_start(
            out=xz[z].ap(), in_=xt.ap()[:, ofs[z] * CH : ofs[z + 1] * CH]
        ).then_inc(s_x[z], 16)
        for v in range(ZSPLIT):
            sy.dma_start(out=zview[z * ZSPLIT + v], in_=ztile.ap()).then_inc(
                s_z[z], 16
            )

    # ---- PE: one projection matmul per chunk (+ selection matmuls) ----
    pe = nc.tensor
    pe.wait_ge(s_w, 16)
    pe.wait_ge(s_id, 2)
    n = 0
    for z in range(ZONES):
        for c in range(quotas[z]):
            if c == 0:
                pe.wait_ge(s_x[z], 16)
            if n >= 4:
                pe.wait_ge(s_src, n - 3)  # pj_ps slot n%4 drained by DVE
            pe.matmul(
                out=pj_ps[n % 4].ap(),
                lhsT=xz[z].ap()[:, c * CH : (c + 1) * CH],
                rhs=wt.ap(),
                start=True,
                stop=True,
            ).then_inc(s_mm, 1)
            if c == 0:
                pe.wait_ge(s_idxf, z + 1)
                if z >= 1:
                    pe.wait_ge(s_idtcp, z)  # idt_ps drained by DVE
                pe.transpose(
                    out=idt_ps.ap(),
                    in_=idxf[z % 2].ap().to_broadcast([CH, CH]),
                    identity=ident.ap(),
                ).then_inc(s_idt, 1)
                pe.wait_ge(s_selv, 2 * (z + 1))  # selm + biased proj ready
                if z >= 1:
                    pe.wait_ge(s_src, ofs[z - 1] + 1)  # acc_ps drained by DVE
                pe.matmul(
                    out=acc_ps.ap(),
                    lhsT=selm[z % 2].ap(),
                    rhs=pj_sb[z % 2].ap(),
                    start=True,
                    stop=True,
                ).then_inc(s_selmm, 1)
            n += 1

    # ---- DVE: ztile memset, bias adds, selection machinery ----
    dv = nc.vector
    dv.memset(ztile.ap(), 0.0).then_inc(s_zt, 1)
    dv.wait_ge(s_b, 16)
    dv.wait_ge(s_ix, 16)
    n = 0
    for z in range(ZONES):
        for c in range(quotas[z]):
            dv.wait_ge(s_mm, n + 1)
            if c == 0:
                dv.tensor_add(
                    out=pj_sb[z % 2].ap(), in0=pj_ps[n % 4].ap(), in1=bt.ap()
                ).then_inc(s_selv, 1)
                dv.tensor_copy(
                    out=idxf[z % 2].ap(), in_=ixt.ap()[:, n : n + 1]
                ).then_inc(s_idxf, 1)
                dv.wait_ge(s_idt, z + 1)
                dv.tensor_copy(out=idt_sb[z % 2].ap(), in_=idt_ps.ap()).then_inc(
                    s_idtcp, 1
                )
                dv.wait_ge(s_idtcp, z + 1)  # own-pipe drain before reading
                dv.wait_ge(s_idxf, z + 1)
                dv.tensor_tensor(
                    out=selm[z % 2].ap(),
                    in0=idxf[z % 2].ap().to_broadcast([CH, CH]),
                    in1=idt_sb[z % 2].ap(),
                    op=mybir.AluOpType.is_equal,
                ).then_inc(s_selv, 1)
                dv.wait_ge(s_selmm, z + 1)
                dv.tensor_copy(
                    out=srcb.ap()[:, n * H : (n + 1) * H], in_=acc_ps.ap()
                ).then_inc(s_src, 1)
            else:
                dv.tensor_add(
                    out=srcb.ap()[:, n * H : (n + 1) * H],
                    in0=pj_ps[n % 4].ap(),
                    in1=bt.ap(),
                ).then_inc(s_src, 1)
            n += 1

    # ---- POOL: identity build, then one indirect scatter per chunk ----
    gp = nc.gpsimd
    gp.memset(ident.ap(), 0.0).then_inc(s_id, 1)
    gp.wait_ge(s_id, 1)
    gp.affine_select(
        out=ident.ap(),
        in_=ident.ap(),
        compare_op=mybir.AluOpType.not_equal,
        fill=1.0,
        base=0,
        pattern=[[-1, CH]],
        channel_multiplier=1,
    ).then_inc(s_id, 1)  # s_id == 2 -> identity ready
    gp.wait_ge(s_ix, 16)
    n = 0
    for z in range(ZONES):
        for c in range(quotas[z]):
            if c == 0:
                gp.wait_ge(s_z[z], 16 * ZSPLIT)  # this zone's rows are zeroed
            gp.indirect_dma_start(
                out=table.ap(),
                out_offset=bass.IndirectOffsetOnAxis(
                    ap=ixt.ap()[:, n : n + 1], axis=0
                ),
                in_=srcb.ap()[:, n * H : (n + 1) * H],
                in_offset=None,
            )._wait_ge(s_src, n + 1).then_inc(s_sc, 16)
            n += 1
    gp.wait_ge(s_sc, 16 * NCH)

    nc.compile()
    return nc


def _prepare(edge_index, edge_attr, n_nodes, n_shards):
    """Bucket edges by (shard, zone).  Chunk 0 of each zone = all
    duplicate-dest groups + singleton filler; remaining singles fill
    chunks 1..  Returns (quotas, xt list [F, NCH*CH], idx list [CH, NCH])
    with per-zone chunk counts maxed over cores.  Table row = TRASH +
    local slot; trash rows < TRASH."""
    N = n_nodes
    R = N // n_shards
    table_real = R * N
    zone_rows = table_real // ZONES
    i = np.asarray(edge_index[0], dtype=np.int64)
    j = np.asarray(edge_index[1], dtype=np.int64)
    valid = (i >= 0) & (i < N) & (j >= 0) & (j < N)
    eids = np.nonzero(valid)[0]
    i = i[eids]
    j = j[eids]
    shard = i // R
    d = (i - shard * R) * N + j
    zone = d // zone_rows

    edge_attr = np.asarray(edge_attr, dtype=np.float32)

    buckets: list = []  # (s, z) -> (edges, dests) ordered: groups then singles
    counts_per_zone = np.zeros((n_shards, ZONES), np.int64)
    for s in range(n_shards):
        for z in range(ZONES):
            m = (shard == s) & (zone == z)
            es, ds = eids[m], d[m]
            o = np.argsort(ds, kind="stable")
            es, ds = es[o], ds[o]
            _, start, counts = np.unique(ds, return_index=True, return_counts=True)
            multi = np.nonzero(counts > 1)[0]
            gsel = np.concatenate(
                [np.arange(start[g], start[g] + counts[g]) for g in multi]
            ) if len(multi) else np.empty(0, np.int64)
            n_grp = len(gsel)
            assert n_grp <= CH, f"{n_grp} duplicate-group edges exceed chunk 0"
            ssel = start[counts == 1]
            order = np.concatenate([gsel, ssel]).astype(np.int64)
            buckets.append((es[order], ds[order]))
            counts_per_zone[s, z] = len(order)

    # per-zone chunk count: maxed over cores (>=1; chunk 0 always exists)
    quotas = tuple(
        int(max(1, -(-int(counts_per_zone[:, z].max()) // CH)))
        for z in range(ZONES)
    )
    NCH = sum(quotas)
    ofs = [0]
    for q in quotas:
        ofs.append(ofs[-1] + q)

    xs, ids = [], []
    bi = 0
    for s in range(n_shards):
        xtp = np.zeros((F, NCH * CH), np.float32)
        idx = np.empty(NCH * CH, np.int64)
        idx[:] = np.arange(NCH * CH) % TRASH  # default: trash rows
        for z in range(ZONES):
            be, bd = buckets[bi]
            bi += 1
            at = ofs[z] * CH
            ne = len(be)
            idx[at : at + ne] = TRASH + bd
            xtp[:, at : at + ne] = edge_attr[be].T
        xs.append(np.ascontiguousarray(xtp))
        ids.append(
            np.ascontiguousarray(
                idx.reshape(NCH, CH).T.astype(np.int32)
            )  # [p, n]
        )
    return quotas, xs, ids


LAST_EXEC_NS = None
LAST_RESULTS = None


def kernel(edge_index, edge_attr, num_nodes, W, b):
    from concourse.bass_utils import run_bass_kernel_spmd

    global LAST_EXEC_NS, LAST_RESULTS
    N = int(num_nodes)
    S = N_CORES
    R = N // S
    table_real = R * N

    quotas, xs, ids = _prepare(edge_index, edge_attr, N, S)
    cfg = _Cfg(n_nodes=N, n_shards=S, quotas=quotas)
    nc = _cache.get(cfg)
    if nc is None:
        nc = _build(cfg)
        _cache[cfg] = nc

    W_np = np.ascontiguousarray(np.asarray(W, dtype=np.float32))
    b_rep = np.ascontiguousarray(
        np.broadcast_to(np.asarray(b, dtype=np.float32), (CH, H))
    )
    in_maps = [
        {"xt": xs[s], "idxb": ids[s], "w": W_np, "brep": b_rep} for s in range(S)
    ]
    trace = bool(int(os.environ.get("EDGE_KERNEL_TRACE", "0")))
    res = run_bass_kernel_spmd(nc, in_maps, core_ids=list(range(S)), trace=trace)
    LAST_EXEC_NS = res.exec_time_ns
    LAST_RESULTS = res
    out = np.concatenate(
        [r["table"][TRASH : TRASH + table_real].reshape(R, N, H) for r in res.results],
        axis=0,
    )
    return out


# revision 17
# speedup vs baseline: 1.0023x; 1.0023x over previous
"""Trainium2 Bass kernel: EdgeFeatureEncoding scatter-add (raw bass).

Computes bias[i, j, :] += edge_attr[e] @ W + b over E edges (i, j),
bias shape (N, N, 8) with N = 4096, E = 131072 -> 512 MiB f32 output.

Strategy (8 NeuronCores, SPMD, hand-rolled semaphores):
- Output rows i are sharded across the 8 cores (512 rows -> 64 MiB each).
- Each shard splits into 16 ZONES with per-zone chunk counts (max over
  cores, so one compiled program fits all cores).
- One HWDGE ring (sync engine) carries, in FIFO order, the constants then
  interleaved (x_z, zero_z) transfers: edge features stream in just ahead
  of each zone's zero-fill, so compute leads the scatter gate.
- Edge features ship pre-transposed ([feat, edge]), so the projection is
  ONE PE matmul per 128-edge chunk straight into [128 edges, 8 heads]
  PSUM (lhsT = xT chunk, rhs = W); DVE adds the bias into the scatter
  source buffer.  No transposes, no PSUM->SBUF relays.
- GpSimd scatters each chunk with one indirect DMA (one dest row per
  partition - HW semantics).  A zone's scatters wait ONLY on that zone's
  own zero-fill semaphore: zero-fill, compute and scatter all pipeline.
- Each zone's chunk 0 carries every duplicate-destination group (plus
  singleton filler); the device group-sums it with the
  is_equal/selection-matrix matmul, so colliding DMA writes all carry the
  identical group sum (singletons pass through the selection matmul
  unchanged).
- Table rows [0, 128) are a trash target for padding edges (sliced off on
  the host); real row d lives at table row 128 + d.
"""

import os
from dataclasses import dataclass

import numpy as np

H = 8  # n_heads
F = 128  # edge feature dim
CH = 128  # edges per chunk (one partition tile / one indirect DMA)
TRASH = 128  # trash rows at the START of the table
N_CORES = 8
ZONES = 16  # zero-fill zones per core
ZSPLIT = 2  # zero-fill DMAs per zone


@dataclass(frozen=True)
class _Cfg:
    n_nodes: int
    n_shards: int
    quotas: tuple  # chunks per zone (chunk 0 of each zone = selection chunk)

    @property
    def rows(self):
        return self.n_nodes // self.n_shards

    @property
    def table_real(self):
        return self.rows * self.n_nodes

    @property
    def zone_rows(self):
        return self.table_real // ZONES

    @property
    def table_rows(self):
        return TRASH + self.table_real


_cache: dict = {}


def _build(cfg: _Cfg):
    import concourse.bacc as bacc
    import concourse.bass as bass
    import concourse.mybir as mybir

    f32 = mybir.dt.float32
    i32 = mybir.dt.int32
    quotas = cfg.quotas
    NCH = sum(quotas)  # total chunks
    ofs = [0]
    for q in quotas:
        ofs.append(ofs[-1] + q)

    nc = bacc.Bacc(
        "TRN2", target_bir_lowering=False, debug=False, num_devices=cfg.n_shards
    )
    # xt[f, (ofs[z] + c)*CH + p] = feature f of edge (zone z, chunk c, row p)
    xt = nc.dram_tensor("xt", [F, NCH * CH], f32, kind="ExternalInput")
    # idxb[p, ofs[z] + c] = dest table row of edge (zone z, chunk c, row p)
    idxb = nc.dram_tensor("idxb", [CH, NCH], i32, kind="ExternalInput")
    w = nc.dram_tensor("w", [F, H], f32, kind="ExternalInput")
    brep = nc.dram_tensor("brep", [CH, H], f32, kind="ExternalInput")
    table = nc.dram_tensor("table", [cfg.table_rows, H], f32, kind="ExternalOutput")

    zcols = cfg.zone_rows * H // (128 * ZSPLIT)  # f32/partition per zero DMA
    zview = table.ap()[TRASH:].rearrange(
        "(zz p x) h -> zz p (x h)", zz=ZONES * ZSPLIT, p=128
    )

    # ---- SBUF / PSUM ----
    ztile = nc.alloc_sbuf_tensor("ztile", [128, zcols], f32)
    wt = nc.alloc_sbuf_tensor("wt", [F, H], f32)
    bt = nc.alloc_sbuf_tensor("bt", [CH, H], f32)
    ixt = nc.alloc_sbuf_tensor("ixt", [CH, NCH], i32)
    ident = nc.alloc_sbuf_tensor("ident", [CH, CH], f32)
    xz = [
        nc.alloc_sbuf_tensor(f"xz{z}", [F, quotas[z] * CH], f32) for z in range(ZONES)
    ]
    srcb = nc.alloc_sbuf_tensor("srcb", [CH, NCH * H], f32)
    idxf = [nc.alloc_sbuf_tensor(f"idxf{i}", [CH, 1], f32) for i in range(2)]
    idt_sb = [nc.alloc_sbuf_tensor(f"idt{i}", [CH, CH], f32) for i in range(2)]
    selm = [nc.alloc_sbuf_tensor(f"selm{i}", [CH, CH], f32) for i in range(2)]
    pj_sb = [nc.alloc_sbuf_tensor(f"pjsb{i}", [CH, H], f32) for i in range(2)]

    pj_ps = [nc.alloc_psum_tensor(f"pj{i}", [CH, H], f32) for i in range(6)]
    idt_ps = nc.alloc_psum_tensor("idtp", [CH, CH], f32)
    acc_ps = nc.alloc_psum_tensor("accp", [CH, H], f32)

    # ---- semaphores ----
    s_zt = nc.alloc_semaphore("s_zt")
    s_w = nc.alloc_semaphore("s_w")
    s_b = nc.alloc_semaphore("s_b")
    s_ix = nc.alloc_semaphore("s_ix")
    s_x = [nc.alloc_semaphore(f"s_x{z}") for z in range(ZONES)]
    s_z = [nc.alloc_semaphore(f"s_z{z}") for z in range(ZONES)]
    s_id = nc.alloc_semaphore("s_id")
    s_mm = nc.alloc_semaphore("s_mm")
    s_src = nc.alloc_semaphore("s_src")
    s_idxf = nc.alloc_semaphore("s_idxf")
    s_idt = nc.alloc_semaphore("s_idt")
    s_idtcp = nc.alloc_semaphore("s_idtcp")
    s_selv = nc.alloc_semaphore("s_selv")
    s_selmm = nc.alloc_semaphore("s_selmm")
    s_sc = nc.alloc_semaphore("s_sc")

    # ---- SYNC: constants, then (x_z, zero_z) interleaved on one ring ----
    sy = nc.sync
    sy.dma_start(out=wt.ap(), in_=w.ap()).then_inc(s_w, 16)
    sy.dma_start(out=bt.ap(), in_=brep.ap()).then_inc(s_b, 16)
    sy.dma_start(out=ixt.ap(), in_=idxb.ap()).then_inc(s_ix, 16)
    sy.wait_ge(s_zt, 1)
    for z in range(ZONES):
        sy.dma_start(
            out=xz[z].ap(), in_=xt.ap()[:, ofs[z] * CH : ofs[z + 1] * CH]
        ).then_inc(s_x[z], 16)
        for v in range(ZSPLIT):
            sy.dma_start(out=zview[z * ZSPLIT + v], in_=ztile.ap()).then_inc(
                s_z[z], 16
            )

    # ---- PE: one projection matmul per chunk (+ selection matmuls) ----
    pe = nc.tensor
    pe.wait_ge(s_w, 16)
    pe.wait_ge(s_id, 2)
    n = 0
    for z in range(ZONES):
        for c in range(quotas[z]):
            if c == 0:
                pe.wait_ge(s_x[z], 16)
            if n >= 6:
                pe.wait_ge(s_src, n - 5)  # pj_ps slot n%6 drained by DVE
            pe.matmul(
                out=pj_ps[n % 6].ap(),
                lhsT=xz[z].ap()[:, c * CH : (c + 1) * CH],
                rhs=wt.ap(),
                start=True,
                stop=True,
            ).then_inc(s_mm, 1)
            if c == 0:
                pe.wait_ge(s_idxf, z + 1)
                if z >= 1:
                    pe.wait_ge(s_idtcp, z)  # idt_ps drained by DVE
                pe.transpose(
                    out=idt_ps.ap(),
                    in_=idxf[z % 2].ap().to_broadcast([CH, CH]),
                    identity=ident.ap(),
                ).then_inc(s_idt, 1)
                pe.wait_ge(s_selv, 2 * (z + 1))  # selm + biased proj ready
                if z >= 1:
                    pe.wait_ge(s_src, ofs[z - 1] + 1)  # acc_ps drained by DVE
                pe.matmul(
                    out=acc_ps.ap(),
                    lhsT=selm[z % 2].ap(),
                    rhs=pj_sb[z % 2].ap(),
                    start=True,
                    stop=True,
                ).then_inc(s_selmm, 1)
            n += 1

    # ---- DVE: ztile memset, bias adds, selection machinery ----
    dv = nc.vector
    dv.memset(ztile.ap(), 0.0).then_inc(s_zt, 1)
    dv.wait_ge(s_b, 16)
    dv.wait_ge(s_ix, 16)
    n = 0
    for z in range(ZONES):
        for c in range(quotas[z]):
            dv.wait_ge(s_mm, n + 1)
            if c == 0:
                dv.tensor_add(
                    out=pj_sb[z % 2].ap(), in0=pj_ps[n % 6].ap(), in1=bt.ap()
                ).then_inc(s_selv, 1)
                dv.tensor_copy(
                    out=idxf[z % 2].ap(), in_=ixt.ap()[:, n : n + 1]
                ).then_inc(s_idxf, 1)
                dv.wait_ge(s_idt, z + 1)
                dv.tensor_copy(out=idt_sb[z % 2].ap(), in_=idt_ps.ap()).then_inc(
                    s_idtcp, 1
                )
                dv.wait_ge(s_idtcp, z + 1)  # own-pipe drain before reading
                dv.wait_ge(s_idxf, z + 1)
                dv.tensor_tensor(
                    out=selm[z % 2].ap(),
                    in0=idxf[z % 2].ap().to_broadcast([CH, CH]),
                    in1=idt_sb[z % 2].ap(),
                    op=mybir.AluOpType.is_equal,
                ).then_inc(s_selv, 1)
                dv.wait_ge(s_selmm, z + 1)
                dv.tensor_copy(
                    out=srcb.ap()[:, n * H : (n + 1) * H], in_=acc_ps.ap()
                ).then_inc(s_src, 1)
            else:
                dv.tensor_add(
                    out=srcb.ap()[:, n * H : (n + 1) * H],
                    in0=pj_ps[n % 6].ap(),
                    in1=bt.ap(),
                ).then_inc(s_src, 1)
            n += 1

    # ---- POOL: identity build, then one indirect scatter per chunk ----
    gp = nc.gpsimd
    gp.memset(ident.ap(), 0.0).then_inc(s_id, 1)
    gp.wait_ge(s_id, 1)
    gp.affine_select(
        out=ident.ap(),
        in_=ident.ap(),
        compare_op=mybir.AluOpType.not_equal,
        fill=1.0,
        base=0,
        pattern=[[-1, CH]],
        channel_multiplier=1,
    ).then_inc(s_id, 1)  # s_id == 2 -> identity ready
    gp.wait_ge(s_ix, 16)
    n = 0
    for z in range(ZONES):
        for c in range(quotas[z]):
            if c == 0:
                gp.wait_ge(s_z[z], 16 * ZSPLIT)  # this zone's rows are zeroed
            gp.indirect_dma_start(
                out=table.ap(),
                out_offset=bass.IndirectOffsetOnAxis(
                    ap=ixt.ap()[:, n : n + 1], axis=0
                ),
                in_=srcb.ap()[:, n * H : (n + 1) * H],
                in_offset=None,
            )._wait_ge(s_src, n + 1).then_inc(s_sc, 16)
            n += 1
    gp.wait_ge(s_sc, 16 * NCH)

    nc.compile()
    return nc


def _prepare(edge_index, edge_attr, n_nodes, n_shards):
    """Bucket edges by (shard, zone).  Chunk 0 of each zone = all
    duplicate-dest groups + singleton filler; remaining singles fill
    chunks 1..  Returns (quotas, xt list [F, NCH*CH], idx list [CH, NCH])
    with per-zone chunk counts maxed over cores.  Table row = TRASH +
    local slot; trash rows < TRASH."""
    N = n_nodes
    R = N // n_shards
    table_real = R * N
    zone_rows = table_real // ZONES
    i = np.asarray(edge_index[0], dtype=np.int64)
    j = np.asarray(edge_index[1], dtype=np.int64)
    valid = (i >= 0) & (i < N) & (j >= 0) & (j < N)
    eids = np.nonzero(valid)[0]
    i = i[eids]
    j = j[eids]
    shard = i // R
    d = (i - shard * R) * N + j
    zone = d // zone_rows

    edge_attr = np.asarray(edge_attr, dtype=np.float32)

    buckets: list = []  # (s, z) -> (edges, dests) ordered: groups then singles
    counts_per_zone = np.zeros((n_shards, ZONES), np.int64)
    for s in range(n_shards):
        for z in range(ZONES):
            m = (shard == s) & (zone == z)
            es, ds = eids[m], d[m]
            o = np.argsort(ds, kind="stable")
            es, ds = es[o], ds[o]
            _, start, counts = np.unique(ds, return_index=True, return_counts=True)
            multi = np.nonzero(counts > 1)[0]
            gsel = np.concatenate(
                [np.arange(start[g], start[g] + counts[g]) for g in multi]
            ) if len(multi) else np.empty(0, np.int64)
            n_grp = len(gsel)
            assert n_grp <= CH, f"{n_grp} duplicate-group edges exceed chunk 0"
            ssel = start[counts == 1]
            order = np.concatenate([gsel, ssel]).astype(np.int64)
            buckets.append((es[order], ds[order]))
            counts_per_zone[s, z] = len(order)

    # per-zone chunk count: maxed over cores (>=1; chunk 0 always exists)
    quotas = tuple(
        int(max(1, -(-int(counts_per_zone[:, z].max()) // CH)))
        for z in range(ZONES)
    )
    NCH = sum(quotas)
    ofs = [0]
    for q in quotas:
        ofs.append(ofs[-1] + q)

    xs, ids = [], []
    bi = 0
    for s in range(n_shards):
        xtp = np.zeros((F, NCH * CH), np.float32)
        idx = np.empty(NCH * CH, np.int64)
        idx[:] = np.arange(NCH * CH) % TRASH  # default: trash rows
        for z in range(ZONES):
            be, bd = buckets[bi]
            bi += 1
            at = ofs[z] * CH
            ne = len(be)
            idx[at : at + ne] = TRASH + bd
            xtp[:, at : at + ne] = edge_attr[be].T
        xs.append(np.ascontiguousarray(xtp))
        ids.append(
            np.ascontiguousarray(
                idx.reshape(NCH, CH).T.astype(np.int32)
            )  # [p, n]
        )
    return quotas, xs, ids


LAST_EXEC_NS = None
LAST_RESULTS = None


def kernel(edge_index, edge_attr, num_nodes, W, b):
    from concourse.bass_utils import run_bass_kernel_spmd

    global LAST_EXEC_NS, LAST_RESULTS
    N = int(num_nodes)
    S = N_CORES
    R = N // S
    table_real = R * N

    quotas, xs, ids = _prepare(edge_index, edge_attr, N, S)
    cfg = _Cfg(n_nodes=N, n_shards=S, quotas=quotas)
    nc = _cache.get(cfg)
    if nc is None:
        nc = _build(cfg)
        _cache[cfg] = nc

    W_np = np.ascontiguousarray(np.asarray(W, dtype=np.float32))
    b_rep = np.ascontiguousarray(
        np.broadcast_to(np.asarray(b, dtype=np.float32), (CH, H))
    )
    in_maps = [
        {"xt": xs[s], "idxb": ids[s], "w": W_np, "brep": b_rep} for s in range(S)
    ]
    trace = bool(int(os.environ.get("EDGE_KERNEL_TRACE", "0")))
    res = run_bass_kernel_spmd(nc, in_maps, core_ids=list(range(S)), trace=trace)
    LAST_EXEC_NS = res.exec_time_ns
    LAST_RESULTS = res
    out = np.concatenate(
        [r["table"][TRASH : TRASH + table_real].reshape(R, N, H) for r in res.results],
        axis=0,
    )
    return out
